# revision 1
# baseline (speedup 1.0000x reference)
"""Trainium2 Bass kernel for the gated-attention MIL pooling layer.

Computes, for x:[256,128,1024], v,u:[1024,512], w:[512,1]:
    h = tanh(x @ v); g = sigmoid(x @ u)
    scores = (h*g) @ w                      # [256,128,1]
    alpha  = softmax(scores, axis=0)        # over the 256 instances

Sharding: data-parallel over the batch axis (128 -> 16 per core, 8 cores).
Each core handles a [4096,1024]x[1024,512] matmul pair + a local softmax
(softmax is over instances, which live entirely on one core).

x is transposed host-side to [in_dim, m] so the Bass kernel can feed the
PE array without on-chip transposes (the contraction dim must sit on SBUF
partitions).  Matmuls run as float32r (full PE rate at moving dim >= 256,
~fp32 precision).

Written in raw Bass (explicit per-engine programs + semaphores): the
walrus build in this container rejects instructions carrying more than
one attached semaphore wait, which rules out Tile-generated sync.  All
waits here are standalone wait_ge instructions.

Startup is DMA-bandwidth-bound (v, u and the first x chunks ~ 8 MB), so
chunks 0 and 1 are streamed per-k-subtile and processed "ko-outer": each
arriving 256 KB piece immediately feeds matmuls for all 8 PSUM
accumulation groups (4 h + 4 g tiles), keeping the PE busy from the
first pieces instead of waiting ~20 us for all weights.
"""

import numpy as np

N_INST, BATCH, IN_DIM, L_DIM = 256, 128, 1024, 512
N_CORES = 8
B_LOC = BATCH // N_CORES            # 16 batch elements per core
M = N_INST * B_LOC                  # 4096 rows per core
P = 128                             # SBUF partitions
KO = IN_DIM // P                    # 8 contraction subtiles
MT = M // P                         # 32 m-tiles per core
MS = 4                              # m-tiles per x DMA chunk
NS = MT // MS                       # 8 DMA chunks

_CACHE = {}


def _build_bass():
    from contextlib import ExitStack

    import concourse.bass as bass
    import concourse.mybir as mybir

    f32 = mybir.dt.float32
    f32r = mybir.dt.float32r
    AF = mybir.ActivationFunctionType
    ALU = mybir.AluOpType

    nc = bass.Bass(
        trn_type="TRN2",
        target_bir_lowering=False,
        debug=False,
        enable_asserts=False,
    )

    xt = nc.dram_tensor("xt", [IN_DIM, M], f32r, kind="ExternalInput").ap()
    v = nc.dram_tensor("v", [IN_DIM, L_DIM], f32r, kind="ExternalInput").ap()
    u = nc.dram_tensor("u", [IN_DIM, L_DIM], f32r, kind="ExternalInput").ap()
    w_rep = nc.dram_tensor("w_rep", [P, L_DIM], f32, kind="ExternalInput").ap()
    # selb[r, c] = (r%16 == c%16): one matmul turns the per-row exp sums
    # into per-batch softmax denominators broadcast back to all 128 rows.
    selb = nc.dram_tensor("selb", [P, P], f32, kind="ExternalInput").ap()
    ident = nc.dram_tensor("ident", [P, P], f32, kind="ExternalInput").ap()
    out = nc.dram_tensor("out", [MT, P], f32, kind="ExternalOutput").ap()

    xt3 = xt.rearrange("(ko p) m -> p ko m", p=P)
    v3 = v.rearrange("(ko p) n -> p ko n", p=P)
    u3 = u.rearrange("(ko p) n -> p ko n", p=P)

    # s_pe tick after the h/g accumulation group of tile t finishes.
    # Chunks 0 and 1 run ko-outer (all four h groups complete, then all
    # four g); steady chunks alternate h/g per tile.
    def pe_h(t):
        return 8 * (t // MS) + t % MS + 1 if t < 2 * MS else 2 * t + 1

    def pe_g(t):
        return 8 * (t // MS) + t % MS + 5 if t < 2 * MS else 2 * t + 2

    # s_act tick after tanh/sigmoid of tile t (ACT always alternates
    # tanh/sigmoid per tile, even while the PE runs chunks 0/1 ko-outer).
    def act_tanh(t):
        return 2 * t + 1

    def act_sig(t):
        return 2 * t + 2

    ctx = ExitStack()
    with ctx:
        v_sb = ctx.enter_context(nc.sbuf_tensor("v_sb", [P, KO, L_DIM], f32r))
        u_sb = ctx.enter_context(nc.sbuf_tensor("u_sb", [P, KO, L_DIM], f32r))
        w_sb = ctx.enter_context(nc.sbuf_tensor("w_sb", [P, L_DIM], f32))
        selb_sb = ctx.enter_context(nc.sbuf_tensor("selb_sb", [P, P], f32))
        id_sb = ctx.enter_context(nc.sbuf_tensor("id_sb", [P, P], f32))
        x_sb = ctx.enter_context(nc.sbuf_tensor("x_sb", [P, 2, KO, MS * P], f32r))
        th_sb = ctx.enter_context(nc.sbuf_tensor("th_sb", [P, MS, L_DIM], f32))
        sg_sb = ctx.enter_context(nc.sbuf_tensor("sg_sb", [P, MS, L_DIM], f32))
        tw_sb = ctx.enter_context(nc.sbuf_tensor("tw_sb", [P, L_DIM], f32))
        z_sb = ctx.enter_context(nc.sbuf_tensor("z_sb", [P, L_DIM], f32))
        S_sb = ctx.enter_context(nc.sbuf_tensor("S_sb", [P, MT], f32))
        E_sb = ctx.enter_context(nc.sbuf_tensor("E_sb", [P, MT], f32))
        rsum_sb = ctx.enter_context(nc.sbuf_tensor("rsum_sb", [P, 1], f32))
        recip_sb = ctx.enter_context(nc.sbuf_tensor("recip_sb", [P, 1], f32))
        alpha_sb = ctx.enter_context(nc.sbuf_tensor("alpha_sb", [P, MT], f32))
        at_sb = ctx.enter_context(nc.sbuf_tensor("at_sb", [MT, P], f32))
        warm_sb = ctx.enter_context(nc.sbuf_tensor("warm_sb", [P, 4], f32))

        # All 8 PSUM banks: 4 h accumulation groups + 4 g groups (slot t%4).
        h_ps = ctx.enter_context(nc.psum_tensor("h_ps", [P, MS, L_DIM], f32))
        g_ps = ctx.enter_context(nc.psum_tensor("g_ps", [P, MS, L_DIM], f32))
        # Epilogue PSUM aliases h banks (dead by then; gated on s_act >= exp).
        rep_ps = h_ps.ap()[:, 1, :1]         # [128, 1] per-batch denominators
        at_ps = h_ps.ap()[:MT, 2, :P]        # [32, 128] transposed alpha

        s_v = [ctx.enter_context(nc.semaphore(f"s_v{k}")) for k in range(KO)]
        s_u = [ctx.enter_context(nc.semaphore(f"s_u{k}")) for k in range(KO)]
        s_x0 = [ctx.enter_context(nc.semaphore(f"s_x0k{k}")) for k in range(KO)]
        s_x1 = [ctx.enter_context(nc.semaphore(f"s_x1k{k}")) for k in range(KO)]
        s_w = ctx.enter_context(nc.semaphore("s_w"))
        s_sel = ctx.enter_context(nc.semaphore("s_sel"))
        s_id = ctx.enter_context(nc.semaphore("s_id"))
        s_x = [ctx.enter_context(nc.semaphore(f"s_x{i}")) for i in range(NS)]
        s_out = ctx.enter_context(nc.semaphore("s_out"))
        s_pe = ctx.enter_context(nc.semaphore("s_pe"))
        s_act = ctx.enter_context(nc.semaphore("s_act"))
        s_dve = ctx.enter_context(nc.semaphore("s_dve"))

        block = ctx.enter_context(nc.Block())

        # Other tick conventions:
        #   s_pe epilogue: denominator matmul -> 65, transpose -> 66.
        #   s_act: exp -> 65.
        #   s_dve: tile t: tw -> 3t+1, z -> 3t+2, reduce -> 3t+3 (96 after
        #          all); epilogue: recip -> 97, alpha -> 98, at copy -> 99.

        @block.sync
        def _(sync):
            # Startup stream: (v, x0, u) per k-subtile for chunk 0, then w,
            # then chunk 1 per k-subtile; steady chunks as whole 2MB DMAs.
            for ko in range(KO):
                sync.dma_start(
                    v_sb.ap()[:, ko, :], v3[:, ko, :]
                ).then_inc(s_v[ko], 16)
                sync.dma_start(
                    x_sb.ap()[:, 0, ko, :], xt3[:, ko, : MS * P]
                ).then_inc(s_x0[ko], 16)
                sync.dma_start(
                    u_sb.ap()[:, ko, :], u3[:, ko, :]
                ).then_inc(s_u[ko], 16)
            sync.dma_start(w_sb.ap(), w_rep[:]).then_inc(s_w, 16)
            for ko in range(KO):
                sync.dma_start(
                    x_sb.ap()[:, 1, ko, :], xt3[:, ko, MS * P : 2 * MS * P]
                ).then_inc(s_x1[ko], 16)
            sync.dma_start(selb_sb.ap(), selb[:]).then_inc(s_sel, 16)
            sync.dma_start(id_sb.ap(), ident[:]).then_inc(s_id, 16)
            for s in range(2, NS):
                # x slot s%2 free once PE finished chunk s-2
                sync.wait_ge(s_pe, 8 * (s - 1))
                sync.dma_start(
                    x_sb.ap()[:, s % 2, :, :],
                    xt3[:, :, s * MS * P : (s + 1) * MS * P],
                ).then_inc(s_x[s], 16)
            sync.wait_ge(s_dve, 3 * MT + 3)
            sync.dma_start(out[:], at_sb.ap()).then_inc(s_out, 16)
            sync.wait_ge(s_out, 16)

        @block.tensor
        def _(tensor):
            # Warm-up: five fp32 broadcast matmuls (~850ns each) keep the PE
            # busy through the DMA-bound startup so the HAM clock gate is at
            # 8/8 when the real matmuls begin.
            c0 = nc.const_aps.aps[(f32, 0.0)]
            c0b = c0.to_broadcast((P, L_DIM))
            for j in range(3):
                nc.tensor.matmul(
                    g_ps.ap()[:1, j, :], c0, c0b, start=True, stop=True
                )
            # ---- chunks 0 and 1: ko-outer over all 8 psum groups ----
            for c in range(2):
                xc = x_sb.ap()[:, c, :, :]
                for ko in range(KO):
                    if c == 0:
                        tensor.wait_ge(s_v[ko], 16)
                        tensor.wait_ge(s_x0[ko], 16)
                    else:
                        tensor.wait_ge(s_x1[ko], 16)
                    for q in range(MS):
                        if c == 1 and ko == 0:
                            # h bank q free once tanh(q) done
                            tensor.wait_ge(s_act, act_tanh(q))
                        mm = nc.tensor.matmul(
                            h_ps.ap()[:, q, :],
                            xc[:, ko, q * P : (q + 1) * P],
                            v_sb.ap()[:, ko, :],
                            start=(ko == 0),
                            stop=(ko == KO - 1),
                        )
                        if ko == KO - 1:
                            mm.then_inc(s_pe, 1)  # ticks 8c + 1..4
                    if c == 0:
                        tensor.wait_ge(s_u[ko], 16)
                    for q in range(MS):
                        if c == 1 and ko == 0:
                            # g bank q free once sigmoid(q) done
                            tensor.wait_ge(s_act, act_sig(q))
                        mm = nc.tensor.matmul(
                            g_ps.ap()[:, q, :],
                            xc[:, ko, q * P : (q + 1) * P],
                            u_sb.ap()[:, ko, :],
                            start=(ko == 0),
                            stop=(ko == KO - 1),
                        )
                        if ko == KO - 1:
                            mm.then_inc(s_pe, 1)  # ticks 8c + 5..8
            # ---- steady chunks ----
            for t in range(2 * MS, MT):
                s, q = divmod(t, MS)
                xq = x_sb.ap()[:, s % 2, :, :]
                # h bank t%4 free once tanh(t-4) done
                tensor.wait_ge(s_act, act_tanh(t - MS))
                if q == 0:
                    tensor.wait_ge(s_x[s], 16)
                for ko in range(KO):
                    mm = nc.tensor.matmul(
                        h_ps.ap()[:, t % MS, :],
                        xq[:, ko, q * P : (q + 1) * P],
                        v_sb.ap()[:, ko, :],
                        start=(ko == 0),
                        stop=(ko == KO - 1),
                    )
                mm.then_inc(s_pe, 1)  # tick 2t+1
                # g bank t%4 free once sigmoid(t-4) done
                tensor.wait_ge(s_act, act_sig(t - MS))
                for ko in range(KO):
                    mm = nc.tensor.matmul(
                        g_ps.ap()[:, t % MS, :],
                        xq[:, ko, q * P : (q + 1) * P],
                        u_sb.ap()[:, ko, :],
                        start=(ko == 0),
                        stop=(ko == KO - 1),
                    )
                mm.then_inc(s_pe, 1)  # tick 2t+2
            # ---- epilogue ----
            tensor.wait_ge(s_sel, 16)
            tensor.wait_ge(s_act, 2 * MT + 1)  # exp/rowsum done; h banks dead
            nc.tensor.matmul(
                rep_ps, selb_sb.ap(), rsum_sb.ap(), start=True, stop=True
            ).then_inc(s_pe, 1)  # -> 65: per-batch sums broadcast to rows
            tensor.wait_ge(s_id, 16)
            tensor.wait_ge(s_dve, 3 * MT + 2)  # alpha ready
            nc.tensor.transpose(at_ps, alpha_sb.ap(), id_sb.ap()).then_inc(
                s_pe, 1
            )  # -> 66

        @block.scalar
        def _(scalar):
            # Dummy activations: pre-load the tanh/sigmoid/exp tables during
            # the DMA-bound startup.
            c0 = nc.const_aps.aps[(f32, 0.0)]
            for j, fn in enumerate((AF.Tanh, AF.Sigmoid, AF.Exp)):
                nc.scalar.activation(warm_sb.ap()[:, j : j + 1], c0, fn)

            def tanh_t(t):
                scalar.wait_ge(s_pe, pe_h(t))
                if t >= MS:
                    scalar.wait_ge(s_dve, 3 * (t - MS) + 1)  # th slot free
                nc.scalar.activation(
                    th_sb.ap()[:, t % MS, :], h_ps.ap()[:, t % MS, :], AF.Tanh
                ).then_inc(s_act, 1)

            def sig_t(t):
                scalar.wait_ge(s_pe, pe_g(t))
                if t >= MS:
                    scalar.wait_ge(s_dve, 3 * (t - MS) + 2)  # sg slot free
                nc.scalar.activation(
                    sg_sb.ap()[:, t % MS, :], g_ps.ap()[:, t % MS, :], AF.Sigmoid
                ).then_inc(s_act, 1)

            for t in range(MT):
                tanh_t(t)
                sig_t(t)
            # Dummy exp BEFORE the final wait: walrus re-emits the exp
            # table load at the sigmoid->exp transition, so trigger it here
            # where it overlaps the DVE tail instead of the critical path.
            nc.scalar.activation(warm_sb.ap()[:, 3:4], c0, AF.Exp)
            # softmax numerators + row sums (no max-subtraction needed:
            # |score| <= sum|w| ~ 28, exp stays well inside fp32 range)
            scalar.wait_ge(s_dve, 3 * MT)  # S complete
            nc.scalar.activation(
                E_sb.ap(), S_sb.ap(), AF.Exp, accum_out=rsum_sb.ap()
            ).then_inc(s_act, 1)  # -> 65

        @block.vector
        def _(vector):
            vector.wait_ge(s_w, 16)
            for t in range(MT):
                vector.wait_ge(s_act, act_tanh(t))
                nc.vector.tensor_tensor(
                    tw_sb.ap(), th_sb.ap()[:, t % MS, :], w_sb.ap(), ALU.mult
                ).then_inc(s_dve, 1)
                vector.wait_ge(s_act, act_sig(t))
                vector.wait_ge(s_dve, 3 * t + 1)  # tw RAW (same-engine order)
                nc.vector.tensor_tensor(
                    z_sb.ap(), tw_sb.ap(), sg_sb.ap()[:, t % MS, :], ALU.mult
                ).then_inc(s_dve, 1)
                vector.wait_ge(s_dve, 3 * t + 2)  # z RAW
                nc.vector.tensor_reduce(
                    S_sb.ap()[:, t : t + 1],
                    z_sb.ap(),
                    axis=mybir.AxisListType.X,
                    op=ALU.add,
                ).then_inc(s_dve, 1)
            # epilogue
            vector.wait_ge(s_pe, 2 * MT + 1)  # rep_ps (denominators) ready
            nc.vector.reciprocal(recip_sb.ap(), rep_ps).then_inc(s_dve, 1)  # 97
            vector.wait_ge(s_act, 2 * MT + 1)  # E ready
            vector.wait_ge(s_dve, 3 * MT + 1)  # recip_sb RAW
            nc.vector.tensor_scalar_mul(
                alpha_sb.ap(), E_sb.ap(), recip_sb.ap()
            ).then_inc(s_dve, 1)  # 98
            vector.wait_ge(s_pe, 2 * MT + 2)  # at_ps ready
            nc.vector.tensor_copy(at_sb.ap(), at_ps).then_inc(s_dve, 1)  # 99

    return nc


def _host_inputs(x, v, u, w):
    """Build the per-core input maps (host-side shard + layout prep)."""
    x = np.asarray(x, dtype=np.float32)
    v = np.ascontiguousarray(np.asarray(v, dtype=np.float32))
    u = np.ascontiguousarray(np.asarray(u, dtype=np.float32))
    w = np.asarray(w, dtype=np.float32).reshape(L_DIM)

    w_rep = np.ascontiguousarray(np.broadcast_to(w, (P, L_DIM)))
    selb = (
        np.arange(P)[:, None] % B_LOC == np.arange(P)[None, :] % B_LOC
    ).astype(np.float32)
    ident = np.eye(P, dtype=np.float32)

    common = {"v": v, "u": u, "w_rep": w_rep, "selb": selb, "ident": ident}
    in_maps = []
    for c in range(N_CORES):
        xc = x[:, c * B_LOC : (c + 1) * B_LOC, :].reshape(M, IN_DIM)
        xtc = np.ascontiguousarray(xc.T)  # [IN_DIM, M]
        in_maps.append({"xt": xtc, **common})
    return in_maps


def kernel(x, v, u, w):
    from concourse.bass_utils import run_bass_kernel_spmd

    if "nc" not in _CACHE:
        _CACHE["nc"] = _build_bass()
    nc = _CACHE["nc"]

    in_maps = _host_inputs(x, v, u, w)
    res = run_bass_kernel_spmd(nc, in_maps, core_ids=list(range(N_CORES)))
    _CACHE["last_result"] = res

    parts = []
    for c in range(N_CORES):
        a = res.results[c]["out"]  # [32, 128], flat index = m = i*16 + b_loc
        parts.append(a.reshape(N_INST, B_LOC))
    full = np.concatenate(parts, axis=1)[:, :, None]
    return np.ascontiguousarray(full.astype(np.float32))



# revision 8
# speedup vs baseline: 1.3843x; 1.3843x over previous
"""Trainium2 Bass kernel for the gated-attention MIL pooling layer.

Computes, for x:[256,128,1024], v,u:[1024,512], w:[512,1]:
    h = tanh(x @ v); g = sigmoid(x @ u)
    scores = (h*g) @ w                      # [256,128,1]
    alpha  = softmax(scores, axis=0)        # over the 256 instances

Sharding: data-parallel over the batch axis (128 -> 16 per core, 8 cores).
Each core computes its 4096 scores on-device; the softmax normalization
(per-batch-element scalar sum / divide over the gathered scores) runs on
the host as part of the unshard step.

Precision split (keeps rel err ~8e-3, gate is 2e-2):
  - h-path (tanh) matmul in bf16: tanh has slope up to 1, needs the bits.
  - g-path (sigmoid) matmul in fp8e4m3 with DoubleRow perf mode (2 fp8
    MACs/cell/cycle, K=256 per instruction): sigmoid's max slope 1/4
    damps the quantization error.
x is shipped twice (bf16 for h, fp8 interleaved for g) = 12.6 MB/core of
DMA vs 109 us of matmul -- still compute-bound.

Per m-tile t (32 tiles of 128 rows):
  PE : 8 bf16 MMs (x_tile stationary, v moving)   -> h_ps bank t%4
       4 fp8-DR MMs (x8 tile stationary, u moving) -> g_ps bank t%4
  ACT: tanh(h_ps)->th, sigmoid(g_ps)->sg
  DVE: tw = th*w ; TTR: z = tw*sg, S[:,t] = sum(z)  (fused reduce)
Output: S_sb [128, 32] scores per core; host does softmax.

Raw Bass (explicit per-engine programs + semaphores): the walrus build
in this container rejects instructions carrying more than one attached
semaphore wait, so all waits are standalone wait_ge instructions.

Startup: ~7 us of NEFF preamble is fixed; after it, w then (v, x-chunk0)
stream per-k-subtile so the PE can start within ~2 us of user code, with
a short burst of dense bf16 warm-up matmuls on a scratch tile to push the
PE HAM clock gate to 8/8 (2.4 GHz) before the real matmuls arrive.
"""

import numpy as np

N_INST, BATCH, IN_DIM, L_DIM = 256, 128, 1024, 512
N_CORES = 8
B_LOC = BATCH // N_CORES            # 16 batch elements per core
M = N_INST * B_LOC                  # 4096 rows per core
P = 128                             # SBUF partitions
KO = IN_DIM // P                    # 8 bf16 contraction subtiles
KS = IN_DIM // (2 * P)              # 4 fp8-DoubleRow contraction subtiles
MT = M // P                         # 32 m-tiles per core
MS = 4                              # m-tiles per x DMA chunk (psum banks)
NS = MT // MS                       # 8 DMA chunks
MSP = MS * P                        # 512 rows per chunk
WARM = 6                            # warm-up matmuls

_CACHE = {}


def _build_bass():
    from contextlib import ExitStack

    import concourse.bass as bass
    import concourse.mybir as mybir

    f32 = mybir.dt.float32
    bf16 = mybir.dt.bfloat16
    f8 = mybir.dt.float8e4
    AF = mybir.ActivationFunctionType
    ALU = mybir.AluOpType
    DR = mybir.MatmulPerfMode.DoubleRow

    nc = bass.Bass(
        trn_type="TRN2",
        target_bir_lowering=False,
        debug=False,
        enable_asserts=False,
    )

    # Host layouts (see _host_inputs):
    #   vt : [KO, P, L]          bf16   vt[ko,p,l] = v[ko*128+p, l]
    #   xb : [NS, KO, P, MSP]    bf16   xb[c,ko,p,m'] = x^T[ko*128+p, c*512+m']
    #   u8 : [KS, P, 2, L]       fp8    u8[ks,p,ko,l] = u[ks*256+ko*128+p, l]
    #   x8 : [NS, KS, P, 2, MSP] fp8    x8[c,ks,p,ko,m'] = x^T[ks*256+ko*128+p, c*512+m']
    #   wr : [P, L]              f32    w replicated across partitions
    vt = nc.dram_tensor("vt", [KO, P, L_DIM], bf16, kind="ExternalInput").ap()
    xb = nc.dram_tensor("xb", [NS, KO, P, MSP], bf16, kind="ExternalInput").ap()
    u8 = nc.dram_tensor("u8", [KS, P, 2, L_DIM], f8, kind="ExternalInput").ap()
    x8 = nc.dram_tensor("x8", [NS, KS, P, 2, MSP], f8, kind="ExternalInput").ap()
    wr = nc.dram_tensor("wr", [P, L_DIM], f32, kind="ExternalInput").ap()
    out = nc.dram_tensor("out", [P, MT], f32, kind="ExternalOutput").ap()

    # s_pe tick after the h/g accumulation group of tile t finishes.
    # Chunk 0 runs ko-outer (4 h groups complete, then 4 g groups);
    # steady tiles alternate h/g.
    def pe_h(t):
        return t + 1 if t < MS else 2 * t + 1

    def pe_g(t):
        return t + 5 if t < MS else 2 * t + 2

    ctx = ExitStack()
    with ctx:
        v_sb = ctx.enter_context(nc.sbuf_tensor("v_sb", [P, KO, L_DIM], bf16))
        u_sb = ctx.enter_context(nc.sbuf_tensor("u_sb", [P, KS, 2, L_DIM], f8))
        xb_sb = ctx.enter_context(nc.sbuf_tensor("xb_sb", [P, 2, KO, MSP], bf16))
        x8_sb = ctx.enter_context(
            nc.sbuf_tensor("x8_sb", [P, 2, KS, 2, MSP], f8)
        )
        w_sb = ctx.enter_context(nc.sbuf_tensor("w_sb", [P, L_DIM], f32))
        th_sb = ctx.enter_context(nc.sbuf_tensor("th_sb", [P, MS, L_DIM], f32))
        sg_sb = ctx.enter_context(nc.sbuf_tensor("sg_sb", [P, MS, L_DIM], f32))
        tw_sb = ctx.enter_context(nc.sbuf_tensor("tw_sb", [P, L_DIM], f32))
        z_sb = ctx.enter_context(nc.sbuf_tensor("z_sb", [P, L_DIM], f32))
        S_sb = ctx.enter_context(nc.sbuf_tensor("S_sb", [P, MT], f32))

        h_ps = ctx.enter_context(nc.psum_tensor("h_ps", [P, MS, L_DIM], f32))
        g_ps = ctx.enter_context(nc.psum_tensor("g_ps", [P, MS, L_DIM], f32))

        s_v = [ctx.enter_context(nc.semaphore(f"s_v{k}")) for k in range(2)]
        s_xb0 = [ctx.enter_context(nc.semaphore(f"s_xb0k{k}")) for k in range(4)]
        s_u = ctx.enter_context(nc.semaphore("s_u"))
        s_w = ctx.enter_context(nc.semaphore("s_w"))
        s_xc = [ctx.enter_context(nc.semaphore(f"s_xc{i}")) for i in range(NS)]
        s_x8c = [ctx.enter_context(nc.semaphore(f"s_x8c{i}")) for i in range(NS)]
        s_pe = ctx.enter_context(nc.semaphore("s_pe"))
        s_act = ctx.enter_context(nc.semaphore("s_act"))
        s_dve = ctx.enter_context(nc.semaphore("s_dve"))
        s_out = ctx.enter_context(nc.semaphore("s_out"))

        block = ctx.enter_context(nc.Block())

        @block.sync
        def _(sync):
            # Startup stream, in PE consumption order.  v/xb chunk-0 are
            # split so the first h matmuls start after ~384 KB: w, v ko0,
            # xb0 ko0, then the rest coarser.
            sync.dma_start(w_sb.ap(), wr[:]).then_inc(s_w, 16)
            sync.dma_start(
                v_sb.ap()[:, 0, :], vt[0]
            ).then_inc(s_v[0], 16)
            sync.dma_start(
                xb_sb.ap()[:, 0, 0, :], xb[0, 0]
            ).then_inc(s_xb0[0], 16)
            sync.dma_start(
                v_sb.ap()[:, 1:, :], vt[1:].rearrange("ko p l -> p ko l")
            ).then_inc(s_v[1], 16)
            sync.dma_start(
                xb_sb.ap()[:, 0, 1:3, :],
                xb[0, 1:3].rearrange("ko p m -> p ko m"),
            ).then_inc(s_xb0[1], 16)
            sync.dma_start(
                xb_sb.ap()[:, 0, 3:5, :],
                xb[0, 3:5].rearrange("ko p m -> p ko m"),
            ).then_inc(s_xb0[2], 16)
            sync.dma_start(
                xb_sb.ap()[:, 0, 5:, :],
                xb[0, 5:].rearrange("ko p m -> p ko m"),
            ).then_inc(s_xb0[3], 16)
            sync.dma_start(
                u_sb.ap(), u8.rearrange("ks p ko l -> p ks ko l")
            ).then_inc(s_u, 16)
            sync.dma_start(
                x8_sb.ap()[:, 0],
                x8[0].rearrange("ks p ko m -> p ks ko m"),
            ).then_inc(s_x8c[0], 16)
            sync.dma_start(
                xb_sb.ap()[:, 1], xb[1].rearrange("ko p m -> p ko m")
            ).then_inc(s_xc[1], 16)
            sync.dma_start(
                x8_sb.ap()[:, 1],
                x8[1].rearrange("ks p ko m -> p ks ko m"),
            ).then_inc(s_x8c[1], 16)
            for s in range(2, NS):
                # x slot s%2 free once PE finished chunk s-2
                sync.wait_ge(s_pe, 8 * s - 8)
                sync.dma_start(
                    xb_sb.ap()[:, s % 2],
                    xb[s].rearrange("ko p m -> p ko m"),
                ).then_inc(s_xc[s], 16)
                sync.dma_start(
                    x8_sb.ap()[:, s % 2],
                    x8[s].rearrange("ks p ko m -> p ks ko m"),
                ).then_inc(s_x8c[s], 16)
            sync.wait_ge(s_dve, 3 * MT)
            sync.dma_start(out[:], S_sb.ap()).then_inc(s_out, 16)
            sync.wait_ge(s_out, 16)

        @block.tensor
        def _(tensor):
            # ---- chunk 0: ko-outer ----
            xc = xb_sb.ap()[:, 0]
            for ko in range(KO):
                if ko == 0:
                    tensor.wait_ge(s_v[0], 16)
                    tensor.wait_ge(s_xb0[0], 16)
                else:
                    tensor.wait_ge(s_v[1], 16)
                    tensor.wait_ge(s_xb0[min((ko + 1) // 2, 3)], 16)
                for q in range(MS):
                    mm = nc.tensor.matmul(
                        h_ps.ap()[:, q, :],
                        xc[:, ko, q * P : (q + 1) * P],
                        v_sb.ap()[:, ko, :],
                        start=(ko == 0),
                        stop=(ko == KO - 1),
                    )
                    if ko == KO - 1:
                        mm.then_inc(s_pe, 1)  # ticks 1..4
            tensor.wait_ge(s_u, 16)
            tensor.wait_ge(s_x8c[0], 16)
            x8c = x8_sb.ap()[:, 0]
            for ks in range(KS):
                for q in range(MS):
                    mm = nc.tensor.matmul(
                        g_ps.ap()[:, q, :],
                        x8c[:, ks, :, q * P : (q + 1) * P],
                        u_sb.ap()[:, ks],
                        start=(ks == 0),
                        stop=(ks == KS - 1),
                        perf_mode=DR,
                    )
                    if ks == KS - 1:
                        mm.then_inc(s_pe, 1)  # ticks 5..8
            # ---- steady tiles ----
            for t in range(MS, MT):
                s, q = divmod(t, MS)
                xq = xb_sb.ap()[:, s % 2]
                x8q = x8_sb.ap()[:, s % 2]
                # h bank t%4 free once tanh(t-4) done
                tensor.wait_ge(s_act, 2 * (t - MS) + 1)
                if q == 0:
                    tensor.wait_ge(s_xc[s], 16)
                for ko in range(KO):
                    mm = nc.tensor.matmul(
                        h_ps.ap()[:, q, :],
                        xq[:, ko, q * P : (q + 1) * P],
                        v_sb.ap()[:, ko, :],
                        start=(ko == 0),
                        stop=(ko == KO - 1),
                    )
                mm.then_inc(s_pe, 1)  # tick 2t+1
                # g bank t%4 free once sigmoid(t-4) done
                tensor.wait_ge(s_act, 2 * (t - MS) + 2)
                if q == 0:
                    tensor.wait_ge(s_x8c[s], 16)
                for ks in range(KS):
                    mm = nc.tensor.matmul(
                        g_ps.ap()[:, q, :],
                        x8q[:, ks, :, q * P : (q + 1) * P],
                        u_sb.ap()[:, ks],
                        start=(ks == 0),
                        stop=(ks == KS - 1),
                        perf_mode=DR,
                    )
                mm.then_inc(s_pe, 1)  # tick 2t+2

        @block.scalar
        def _(scalar):
            for t in range(MT):
                scalar.wait_ge(s_pe, pe_h(t))
                if t >= MS:
                    scalar.wait_ge(s_dve, 3 * (t - MS) + 1)  # th slot free
                nc.scalar.activation(
                    th_sb.ap()[:, t % MS, :], h_ps.ap()[:, t % MS, :], AF.Tanh
                ).then_inc(s_act, 1)  # tick 2t+1
                scalar.wait_ge(s_pe, pe_g(t))
                if t >= MS:
                    scalar.wait_ge(s_dve, 3 * (t - MS) + 2)  # sg slot free
                nc.scalar.activation(
                    sg_sb.ap()[:, t % MS, :], g_ps.ap()[:, t % MS, :], AF.Sigmoid
                ).then_inc(s_act, 1)  # tick 2t+2

        @block.vector
        def _(vector):
            vector.wait_ge(s_w, 16)
            for t in range(MT):
                vector.wait_ge(s_act, 2 * t + 1)
                if t:
                    vector.wait_ge(s_dve, 3 * t - 1)  # tw WAR (same engine)
                nc.vector.tensor_tensor(
                    tw_sb.ap(), th_sb.ap()[:, t % MS, :], w_sb.ap(), ALU.mult
                ).then_inc(s_dve, 1)  # tick 3t+1
                vector.wait_ge(s_act, 2 * t + 2)
                vector.wait_ge(s_dve, 3 * t + 1)  # tw RAW (same engine)
                nc.vector.tensor_tensor(
                    z_sb.ap(), tw_sb.ap(), sg_sb.ap()[:, t % MS, :], ALU.mult
                ).then_inc(s_dve, 1)  # tick 3t+2
                vector.wait_ge(s_dve, 3 * t + 2)  # z RAW (same engine)
                nc.vector.tensor_reduce(
                    S_sb.ap()[:, t : t + 1],
                    z_sb.ap(),
                    axis=mybir.AxisListType.X,
                    op=ALU.add,
                ).then_inc(s_dve, 1)  # tick 3t+3

    return nc


def _host_inputs(x, v, u, w):
    """Build the per-core input maps (host-side shard + layout prep)."""
    import ml_dtypes

    bf16 = ml_dtypes.bfloat16
    f8 = ml_dtypes.float8_e4m3fn

    x = np.asarray(x, dtype=np.float32)
    v = np.asarray(v, dtype=np.float32)
    u = np.asarray(u, dtype=np.float32)
    w = np.asarray(w, dtype=np.float32).reshape(L_DIM)

    # vt[ko, p, l] = v[ko*128+p, l]
    vt = np.ascontiguousarray(v.reshape(KO, P, L_DIM).astype(bf16))
    # u8[ks, p, ko, l] = u[ks*256+ko*128+p, l]
    u8 = np.ascontiguousarray(
        u.reshape(KS, 2, P, L_DIM).transpose(0, 2, 1, 3).astype(f8)
    )
    wr = np.ascontiguousarray(np.broadcast_to(w, (P, L_DIM)))

    common = {"vt": vt, "u8": u8, "wr": wr}
    in_maps = []
    for c in range(N_CORES):
        xc = x[:, c * B_LOC : (c + 1) * B_LOC, :].reshape(M, IN_DIM)
        xt = np.ascontiguousarray(xc.T)  # [IN_DIM, M] f32
        # xb[c, ko, p, m'] = xt[ko*128+p, c*512+m']
        xbc = np.ascontiguousarray(
            xt.reshape(KO, P, NS, MSP).transpose(2, 0, 1, 3).astype(bf16)
        )
        # x8[c, ks, p, ko, m'] = xt[ks*256+ko*128+p, c*512+m']
        x8c = np.ascontiguousarray(
            xt.reshape(KS, 2, P, NS, MSP).transpose(3, 0, 2, 1, 4).astype(f8)
        )
        in_maps.append({"xb": xbc, "x8": x8c, **common})
    return in_maps


def kernel(x, v, u, w):
    from concourse.bass_utils import run_bass_kernel_spmd

    if "nc" not in _CACHE:
        _CACHE["nc"] = _build_bass()
    nc = _CACHE["nc"]

    in_maps = _host_inputs(x, v, u, w)
    res = run_bass_kernel_spmd(nc, in_maps, core_ids=list(range(N_CORES)))
    _CACHE["last_result"] = res

    # Gather scores and finish the softmax (over instances) on the host.
    parts = []
    for c in range(N_CORES):
        S = res.results[c]["out"]  # [128, 32], score of row m = t*128 + r
        parts.append(S.T.reshape(M).reshape(N_INST, B_LOC))
    scores = np.concatenate(parts, axis=1).astype(np.float64)  # [256, 128]
    scores -= scores.max(axis=0, keepdims=True)
    e = np.exp(scores)
    alpha = e / e.sum(axis=0, keepdims=True)
    return np.ascontiguousarray(alpha[:, :, None].astype(np.float32))


# revision 15
# speedup vs baseline: 1.4003x; 1.0116x over previous
"""Trainium2 Bass kernel for the gated-attention MIL pooling layer.

Computes, for x:[256,128,1024], v,u:[1024,512], w:[512,1]:
    h = tanh(x @ v); g = sigmoid(x @ u)
    scores = (h*g) @ w                      # [256,128,1]
    alpha  = softmax(scores, axis=0)        # over the 256 instances

Sharding: data-parallel over the batch axis (128 -> 16 per core, 8 cores).
Each core computes its 4096 scores on-device; the softmax normalization
(per-batch-element scalar sum / divide over the gathered scores) runs on
the host as part of the unshard step.

Precision split (keeps rel err ~8e-3, gate is 2e-2):
  - h-path (tanh) matmul in bf16: tanh has slope up to 1, needs the bits.
  - g-path (sigmoid) matmul in fp8e4m3 with DoubleRow perf mode (2 fp8
    MACs/cell/cycle, K=256 per instruction): sigmoid's max slope 1/4
    damps the quantization error.
x is shipped twice (bf16 for h, fp8 interleaved for g) = 12.6 MB/core of
DMA vs 109 us of matmul -- still compute-bound.

Per m-tile t (32 tiles of 128 rows):
  PE : 8 bf16 MMs (x_tile stationary, v moving)   -> h_ps bank t%4
       4 fp8-DR MMs (x8 tile stationary, u moving) -> g_ps bank t%4
  ACT: tanh(h_ps)->th, sigmoid(g_ps)->sg
  DVE: tw = th*w ; TTR: z = tw*sg, S[:,t] = sum(z)  (fused reduce)
Output: S_sb [128, 32] scores per core; host does softmax.

Raw Bass (explicit per-engine programs + semaphores): the walrus build
in this container rejects instructions carrying more than one attached
semaphore wait, so all waits are standalone wait_ge instructions.

Startup: ~7 us of NEFF preamble is fixed; after it, w then (v, x-chunk0)
stream per-k-subtile so the PE can start within ~2 us of user code, with
a short burst of dense bf16 warm-up matmuls on a scratch tile to push the
PE HAM clock gate to 8/8 (2.4 GHz) before the real matmuls arrive.
"""

import numpy as np

N_INST, BATCH, IN_DIM, L_DIM = 256, 128, 1024, 512
N_CORES = 8
B_LOC = BATCH // N_CORES            # 16 batch elements per core
M = N_INST * B_LOC                  # 4096 rows per core
P = 128                             # SBUF partitions
KO = IN_DIM // P                    # 8 bf16 contraction subtiles
KS = IN_DIM // (2 * P)              # 4 fp8-DoubleRow contraction subtiles
MT = M // P                         # 32 m-tiles per core
MS = 4                              # m-tiles per x DMA chunk (psum banks)
NS = MT // MS                       # 8 DMA chunks
MSP = MS * P                        # 512 rows per chunk
WARM = 16                           # warm-up matmuls

_CACHE = {}


def _build_bass():
    from contextlib import ExitStack

    import concourse.bass as bass
    import concourse.mybir as mybir

    f32 = mybir.dt.float32
    bf16 = mybir.dt.bfloat16
    f8 = mybir.dt.float8e4
    AF = mybir.ActivationFunctionType
    ALU = mybir.AluOpType
    DR = mybir.MatmulPerfMode.DoubleRow

    nc = bass.Bass(
        trn_type="TRN2",
        target_bir_lowering=False,
        debug=False,
        enable_asserts=False,
    )

    # Host layouts (see _host_inputs):
    #   vt : [KO, P, L]          bf16   vt[ko,p,l] = v[ko*128+p, l]
    #   xb : [NS, KO, P, MSP]    bf16   xb[c,ko,p,m'] = x^T[ko*128+p, c*512+m']
    #   u8 : [KS, P, 2, L]       fp8    u8[ks,p,ko,l] = u[ks*256+ko*128+p, l]
    #   x8 : [NS, KS, P, 2, MSP] fp8    x8[c,ks,p,ko,m'] = x^T[ks*256+ko*128+p, c*512+m']
    #   wr : [P, L]              f32    w replicated across partitions
    vt = nc.dram_tensor("vt", [KO, P, L_DIM], bf16, kind="ExternalInput").ap()
    xb = nc.dram_tensor("xb", [NS, KO, P, MSP], bf16, kind="ExternalInput").ap()
    u8 = nc.dram_tensor("u8", [KS, P, 2, L_DIM], f8, kind="ExternalInput").ap()
    x8 = nc.dram_tensor("x8", [NS, KS, P, 2, MSP], f8, kind="ExternalInput").ap()
    wr = nc.dram_tensor("wr", [P, L_DIM], f32, kind="ExternalInput").ap()
    wz = nc.dram_tensor("wz", [P, 64], bf16, kind="ExternalInput").ap()
    out = nc.dram_tensor("out", [P, MT], f32, kind="ExternalOutput").ap()

    # s_pe tick after the h/g accumulation group of tile t finishes.
    # Chunk 0 runs ko-outer (4 h groups complete, then 4 g groups);
    # steady tiles alternate h/g.
    def pe_h(t):
        return t + 1 if t < MS else 2 * t + 1

    def pe_g(t):
        return t + 5 if t < MS else 2 * t + 2

    ctx = ExitStack()
    with ctx:
        v_sb = ctx.enter_context(nc.sbuf_tensor("v_sb", [P, KO, L_DIM], bf16))
        u_sb = ctx.enter_context(nc.sbuf_tensor("u_sb", [P, KS, 2, L_DIM], f8))
        xb_sb = ctx.enter_context(nc.sbuf_tensor("xb_sb", [P, 2, KO, MSP], bf16))
        x8_sb = ctx.enter_context(
            nc.sbuf_tensor("x8_sb", [P, 2, KS, 2, MSP], f8)
        )
        w_sb = ctx.enter_context(nc.sbuf_tensor("w_sb", [P, L_DIM], f32))
        th_sb = ctx.enter_context(nc.sbuf_tensor("th_sb", [P, MS, L_DIM], f32))
        sg_sb = ctx.enter_context(nc.sbuf_tensor("sg_sb", [P, MS, L_DIM], f32))
        tw_sb = ctx.enter_context(nc.sbuf_tensor("tw_sb", [P, L_DIM], f32))
        z_sb = ctx.enter_context(nc.sbuf_tensor("z_sb", [P, L_DIM], f32))
        S_sb = ctx.enter_context(nc.sbuf_tensor("S_sb", [P, MT], f32))
        wz_sb = ctx.enter_context(nc.sbuf_tensor("wz_sb", [P, 64], bf16))

        h_ps = ctx.enter_context(nc.psum_tensor("h_ps", [P, MS, L_DIM], f32))
        g_ps = ctx.enter_context(nc.psum_tensor("g_ps", [P, MS, L_DIM], f32))

        s_wz = ctx.enter_context(nc.semaphore("s_wz"))
        s_v = [ctx.enter_context(nc.semaphore(f"s_v{k}")) for k in range(4)]
        s_xb0 = [ctx.enter_context(nc.semaphore(f"s_xb0k{k}")) for k in range(4)]
        s_u = ctx.enter_context(nc.semaphore("s_u"))
        s_w = ctx.enter_context(nc.semaphore("s_w"))
        s_xc = [ctx.enter_context(nc.semaphore(f"s_xc{i}")) for i in range(NS)]
        s_x8c = [ctx.enter_context(nc.semaphore(f"s_x8c{i}")) for i in range(NS)]
        s_pe = ctx.enter_context(nc.semaphore("s_pe"))
        s_act = ctx.enter_context(nc.semaphore("s_act"))
        s_dve = ctx.enter_context(nc.semaphore("s_dve"))
        s_out = ctx.enter_context(nc.semaphore("s_out"))

        block = ctx.enter_context(nc.Block())

        @block.sync
        def _(sync):
            # Startup stream, in PE consumption order: warm tile first,
            # then interleaved (v ko-piece, xb chunk-0 ko-piece) pairs so
            # the first h matmuls start after ~272 KB and never starve.
            sync.dma_start(wz_sb.ap(), wz[:]).then_inc(s_wz, 16)
            sync.dma_start(
                v_sb.ap()[:, 0, :], vt[0]
            ).then_inc(s_v[0], 16)
            sync.dma_start(
                xb_sb.ap()[:, 0, 0, :], xb[0, 0]
            ).then_inc(s_xb0[0], 16)
            sync.dma_start(
                v_sb.ap()[:, 1:3, :], vt[1:3].rearrange("ko p l -> p ko l")
            ).then_inc(s_v[1], 16)
            sync.dma_start(
                xb_sb.ap()[:, 0, 1:3, :],
                xb[0, 1:3].rearrange("ko p m -> p ko m"),
            ).then_inc(s_xb0[1], 16)
            sync.dma_start(
                v_sb.ap()[:, 3:5, :], vt[3:5].rearrange("ko p l -> p ko l")
            ).then_inc(s_v[2], 16)
            sync.dma_start(
                xb_sb.ap()[:, 0, 3:5, :],
                xb[0, 3:5].rearrange("ko p m -> p ko m"),
            ).then_inc(s_xb0[2], 16)
            sync.dma_start(
                v_sb.ap()[:, 5:, :], vt[5:].rearrange("ko p l -> p ko l")
            ).then_inc(s_v[3], 16)
            sync.dma_start(
                xb_sb.ap()[:, 0, 5:, :],
                xb[0, 5:].rearrange("ko p m -> p ko m"),
            ).then_inc(s_xb0[3], 16)
            sync.dma_start(w_sb.ap(), wr[:]).then_inc(s_w, 16)
            sync.dma_start(
                u_sb.ap(), u8.rearrange("ks p ko l -> p ks ko l")
            ).then_inc(s_u, 16)
            sync.dma_start(
                x8_sb.ap()[:, 0],
                x8[0].rearrange("ks p ko m -> p ks ko m"),
            ).then_inc(s_x8c[0], 16)
            sync.dma_start(
                xb_sb.ap()[:, 1], xb[1].rearrange("ko p m -> p ko m")
            ).then_inc(s_xc[1], 16)
            sync.dma_start(
                x8_sb.ap()[:, 1],
                x8[1].rearrange("ks p ko m -> p ks ko m"),
            ).then_inc(s_x8c[1], 16)
            for s in range(2, NS):
                # x slot s%2 free once PE finished chunk s-2
                sync.wait_ge(s_pe, 8 * s - 8)
                sync.dma_start(
                    xb_sb.ap()[:, s % 2],
                    xb[s].rearrange("ko p m -> p ko m"),
                ).then_inc(s_xc[s], 16)
                sync.dma_start(
                    x8_sb.ap()[:, s % 2],
                    x8[s].rearrange("ks p ko m -> p ks ko m"),
                ).then_inc(s_x8c[s], 16)
            sync.wait_ge(s_dve, 3 * MT)
            sync.dma_start(out[:], S_sb.ap()).then_inc(s_out, 16)
            sync.wait_ge(s_out, 16)

        @block.tensor
        def _(tensor):
            # Dense warm-up matmuls on the zero tile: keep the PE busy
            # through the DMA-bound startup so the HAM clock gate reaches
            # 8/8 (2.4 GHz) around when the real matmuls begin.
            tensor.wait_ge(s_wz, 16)
            for j in range(WARM):
                nc.tensor.matmul(
                    g_ps.ap()[:64, 3, :64],
                    wz_sb.ap()[:, :64],
                    wz_sb.ap(),
                    start=True,
                    stop=True,
                )
            # ---- chunk 0: ko-outer ----
            xc = xb_sb.ap()[:, 0]
            for ko in range(KO):
                kp = min((ko + 1) // 2, 3)
                tensor.wait_ge(s_v[kp], 16)
                tensor.wait_ge(s_xb0[kp], 16)
                for q in range(MS):
                    mm = nc.tensor.matmul(
                        h_ps.ap()[:, q, :],
                        xc[:, ko, q * P : (q + 1) * P],
                        v_sb.ap()[:, ko, :],
                        start=(ko == 0),
                        stop=(ko == KO - 1),
                    )
                    if ko == KO - 1:
                        mm.then_inc(s_pe, 1)  # ticks 1..4
            tensor.wait_ge(s_u, 16)
            tensor.wait_ge(s_x8c[0], 16)
            x8c = x8_sb.ap()[:, 0]
            for ks in range(KS):
                for q in range(MS):
                    mm = nc.tensor.matmul(
                        g_ps.ap()[:, q, :],
                        x8c[:, ks, :, q * P : (q + 1) * P],
                        u_sb.ap()[:, ks],
                        start=(ks == 0),
                        stop=(ks == KS - 1),
                        perf_mode=DR,
                    )
                    if ks == KS - 1:
                        mm.then_inc(s_pe, 1)  # ticks 5..8
            # ---- steady tiles ----
            for t in range(MS, MT):
                s, q = divmod(t, MS)
                xq = xb_sb.ap()[:, s % 2]
                x8q = x8_sb.ap()[:, s % 2]
                # h bank t%4 free once tanh(t-4) done
                tensor.wait_ge(s_act, 2 * (t - MS) + 1)
                if q == 0:
                    tensor.wait_ge(s_xc[s], 16)
                for ko in range(KO):
                    mm = nc.tensor.matmul(
                        h_ps.ap()[:, q, :],
                        xq[:, ko, q * P : (q + 1) * P],
                        v_sb.ap()[:, ko, :],
                        start=(ko == 0),
                        stop=(ko == KO - 1),
                    )
                mm.then_inc(s_pe, 1)  # tick 2t+1
                # g bank t%4 free once sigmoid(t-4) done
                tensor.wait_ge(s_act, 2 * (t - MS) + 2)
                if q == 0:
                    tensor.wait_ge(s_x8c[s], 16)
                for ks in range(KS):
                    mm = nc.tensor.matmul(
                        g_ps.ap()[:, q, :],
                        x8q[:, ks, :, q * P : (q + 1) * P],
                        u_sb.ap()[:, ks],
                        start=(ks == 0),
                        stop=(ks == KS - 1),
                        perf_mode=DR,
                    )
                mm.then_inc(s_pe, 1)  # tick 2t+2

        @block.scalar
        def _(scalar):
            for t in range(MT):
                scalar.wait_ge(s_pe, pe_h(t))
                if t >= MS:
                    scalar.wait_ge(s_dve, 3 * (t - MS) + 1)  # th slot free
                nc.scalar.activation(
                    th_sb.ap()[:, t % MS, :], h_ps.ap()[:, t % MS, :], AF.Tanh
                ).then_inc(s_act, 1)  # tick 2t+1
                scalar.wait_ge(s_pe, pe_g(t))
                if t >= MS:
                    scalar.wait_ge(s_dve, 3 * (t - MS) + 2)  # sg slot free
                nc.scalar.activation(
                    sg_sb.ap()[:, t % MS, :], g_ps.ap()[:, t % MS, :], AF.Sigmoid
                ).then_inc(s_act, 1)  # tick 2t+2

        @block.vector
        def _(vector):
            vector.wait_ge(s_w, 16)
            for t in range(MT):
                vector.wait_ge(s_act, 2 * t + 1)
                if t:
                    vector.wait_ge(s_dve, 3 * t - 1)  # tw WAR (same engine)
                nc.vector.tensor_tensor(
                    tw_sb.ap(), th_sb.ap()[:, t % MS, :], w_sb.ap(), ALU.mult
                ).then_inc(s_dve, 1)  # tick 3t+1
                vector.wait_ge(s_act, 2 * t + 2)
                vector.wait_ge(s_dve, 3 * t + 1)  # tw RAW (same engine)
                nc.vector.tensor_tensor(
                    z_sb.ap(), tw_sb.ap(), sg_sb.ap()[:, t % MS, :], ALU.mult
                ).then_inc(s_dve, 1)  # tick 3t+2
                vector.wait_ge(s_dve, 3 * t + 2)  # z RAW (same engine)
                nc.vector.tensor_reduce(
                    S_sb.ap()[:, t : t + 1],
                    z_sb.ap(),
                    axis=mybir.AxisListType.X,
                    op=ALU.add,
                ).then_inc(s_dve, 1)  # tick 3t+3

    return nc


def _host_inputs(x, v, u, w):
    """Build the per-core input maps (host-side shard + layout prep)."""
    import ml_dtypes

    bf16 = ml_dtypes.bfloat16
    f8 = ml_dtypes.float8_e4m3fn

    x = np.asarray(x, dtype=np.float32)
    v = np.asarray(v, dtype=np.float32)
    u = np.asarray(u, dtype=np.float32)
    w = np.asarray(w, dtype=np.float32).reshape(L_DIM)

    # vt[ko, p, l] = v[ko*128+p, l]
    vt = np.ascontiguousarray(v.reshape(KO, P, L_DIM).astype(bf16))
    # u8[ks, p, ko, l] = u[ks*256+ko*128+p, l]
    u8 = np.ascontiguousarray(
        u.reshape(KS, 2, P, L_DIM).transpose(0, 2, 1, 3).astype(f8)
    )
    wr = np.ascontiguousarray(np.broadcast_to(w, (P, L_DIM)))
    wz = np.zeros((P, 64), dtype=bf16)

    common = {"vt": vt, "u8": u8, "wr": wr, "wz": wz}
    in_maps = []
    for c in range(N_CORES):
        xc = x[:, c * B_LOC : (c + 1) * B_LOC, :].reshape(M, IN_DIM)
        xt = np.ascontiguousarray(xc.T)  # [IN_DIM, M] f32
        # xb[c, ko, p, m'] = xt[ko*128+p, c*512+m']
        xbc = np.ascontiguousarray(
            xt.reshape(KO, P, NS, MSP).transpose(2, 0, 1, 3).astype(bf16)
        )
        # x8[c, ks, p, ko, m'] = xt[ks*256+ko*128+p, c*512+m']
        x8c = np.ascontiguousarray(
            xt.reshape(KS, 2, P, NS, MSP).transpose(3, 0, 2, 1, 4).astype(f8)
        )
        in_maps.append({"xb": xbc, "x8": x8c, **common})
    return in_maps


def kernel(x, v, u, w):
    from concourse.bass_utils import run_bass_kernel_spmd

    if "nc" not in _CACHE:
        _CACHE["nc"] = _build_bass()
    nc = _CACHE["nc"]

    in_maps = _host_inputs(x, v, u, w)
    res = run_bass_kernel_spmd(nc, in_maps, core_ids=list(range(N_CORES)))
    _CACHE["last_result"] = res

    # Gather scores and finish the softmax (over instances) on the host.
    parts = []
    for c in range(N_CORES):
        S = res.results[c]["out"]  # [128, 32], score of row m = t*128 + r
        parts.append(S.T.reshape(M).reshape(N_INST, B_LOC))
    scores = np.concatenate(parts, axis=1).astype(np.float64)  # [256, 128]
    scores -= scores.max(axis=0, keepdims=True)
    e = np.exp(scores)
    alpha = e / e.sum(axis=0, keepdims=True)
    return np.ascontiguousarray(alpha[:, :, None].astype(np.float32))


# revision 21
# speedup vs baseline: 1.4307x; 1.0216x over previous
"""Trainium2 Bass kernel for the gated-attention MIL pooling layer.

Computes, for x:[256,128,1024], v,u:[1024,512], w:[512,1]:
    h = tanh(x @ v); g = sigmoid(x @ u)
    scores = (h*g) @ w                      # [256,128,1]
    alpha  = softmax(scores, axis=0)        # over the 256 instances

Sharding: data-parallel over the batch axis (128 -> 16 per core, 8 cores).
Each core computes its 4096 scores on-device; the softmax normalization
(per-batch-element scalar sum / divide over the gathered scores) runs on
the host as part of the unshard step.

Precision split (keeps rel err ~8e-3, gate is 2e-2):
  - h-path (tanh) matmul in bf16: tanh has slope up to 1, needs the bits.
  - g-path (sigmoid) matmul in fp8e4m3 with DoubleRow perf mode (2 fp8
    MACs/cell/cycle, K=256 per instruction): sigmoid's max slope 1/4
    damps the quantization error.
x is shipped twice (bf16 for h, fp8 interleaved for g) = 12.6 MB/core of
DMA vs 109 us of matmul -- still compute-bound.

Per m-tile t (32 tiles of 128 rows):
  PE : 8 bf16 MMs (x_tile stationary, v moving)   -> h_ps bank t%4
       4 fp8-DR MMs (x8 tile stationary, u moving) -> g_ps bank t%4
  ACT: tanh(h_ps)->th, sigmoid(g_ps)->sg
  DVE: tw = th*w ; TTR: z = tw*sg, S[:,t] = sum(z)  (fused reduce)
Output: S_sb [128, 32] scores per core; host does softmax.

Raw Bass (explicit per-engine programs + semaphores): the walrus build
in this container rejects instructions carrying more than one attached
semaphore wait, so all waits are standalone wait_ge instructions.

Startup: ~7 us of NEFF preamble is fixed; after it, w then (v, x-chunk0)
stream per-k-subtile so the PE can start within ~2 us of user code, with
a short burst of dense bf16 warm-up matmuls on a scratch tile to push the
PE HAM clock gate to 8/8 (2.4 GHz) before the real matmuls arrive.
"""

import numpy as np

N_INST, BATCH, IN_DIM, L_DIM = 256, 128, 1024, 512
N_CORES = 8
B_LOC = BATCH // N_CORES            # 16 batch elements per core
M = N_INST * B_LOC                  # 4096 rows per core
P = 128                             # SBUF partitions
KO = IN_DIM // P                    # 8 bf16 contraction subtiles
KS = IN_DIM // (2 * P)              # 4 fp8-DoubleRow contraction subtiles
MT = M // P                         # 32 m-tiles per core
MS = 4                              # m-tiles per x DMA chunk (psum banks)
NS = MT // MS                       # 8 DMA chunks
MSP = MS * P                        # 512 rows per chunk
WARM = 2                            # warm-up matmul calls (fp32, ~2us each)

_CACHE = {}


def _build_bass():
    from contextlib import ExitStack

    import concourse.bass as bass
    import concourse.mybir as mybir

    f32 = mybir.dt.float32
    bf16 = mybir.dt.bfloat16
    f8 = mybir.dt.float8e4
    AF = mybir.ActivationFunctionType
    ALU = mybir.AluOpType
    DR = mybir.MatmulPerfMode.DoubleRow

    nc = bass.Bass(
        trn_type="TRN2",
        target_bir_lowering=False,
        debug=False,
        enable_asserts=False,
    )

    # Host layouts (see _host_inputs):
    #   vt : [KO, P, L]          bf16   vt[ko,p,l] = v[ko*128+p, l]
    #   xb : [NS, KO, P, MSP]    bf16   xb[c,ko,p,m'] = x^T[ko*128+p, c*512+m']
    #   u8 : [KS, P, 2, L]       fp8    u8[ks,p,ko,l] = u[ks*256+ko*128+p, l]
    #   x8 : [NS, KS, P, 2, MSP] fp8    x8[c,ks,p,ko,m'] = x^T[ks*256+ko*128+p, c*512+m']
    #   wr : [P, L]              f32    w replicated across partitions
    vt = nc.dram_tensor("vt", [KO, P, L_DIM], bf16, kind="ExternalInput").ap()
    xb = nc.dram_tensor("xb", [NS, KO, P, MSP], bf16, kind="ExternalInput").ap()
    u8 = nc.dram_tensor("u8", [KS, P, 2, L_DIM], f8, kind="ExternalInput").ap()
    x8 = nc.dram_tensor("x8", [NS, KS, P, 2, MSP], f8, kind="ExternalInput").ap()
    wr = nc.dram_tensor("wr", [P, L_DIM], f32, kind="ExternalInput").ap()
    out = nc.dram_tensor("out", [P, MT], f32, kind="ExternalOutput").ap()

    # s_pe tick after the h/g accumulation group of tile t finishes.
    # Chunk 0 runs ko-outer (4 h groups complete, then 4 g groups);
    # steady tiles alternate h/g.
    def pe_h(t):
        return t + 1 if t < MS else 2 * t + 1

    def pe_g(t):
        return t + 5 if t < MS else 2 * t + 2

    ctx = ExitStack()
    with ctx:
        v_sb = ctx.enter_context(nc.sbuf_tensor("v_sb", [P, KO, L_DIM], bf16))
        u_sb = ctx.enter_context(nc.sbuf_tensor("u_sb", [P, KS, 2, L_DIM], f8))
        xb_sb = ctx.enter_context(nc.sbuf_tensor("xb_sb", [P, 2, KO, MSP], bf16))
        x8_sb = ctx.enter_context(
            nc.sbuf_tensor("x8_sb", [P, 2, KS, 2, MSP], f8)
        )
        w_sb = ctx.enter_context(nc.sbuf_tensor("w_sb", [P, L_DIM], f32))
        th_sb = ctx.enter_context(nc.sbuf_tensor("th_sb", [P, MS, L_DIM], f32))
        sg_sb = ctx.enter_context(nc.sbuf_tensor("sg_sb", [P, MS, L_DIM], f32))
        tw_sb = ctx.enter_context(nc.sbuf_tensor("tw_sb", [P, L_DIM], f32))
        z_sb = ctx.enter_context(nc.sbuf_tensor("z_sb", [P, L_DIM], f32))
        S_sb = ctx.enter_context(nc.sbuf_tensor("S_sb", [P, MT], f32))

        h_ps = ctx.enter_context(nc.psum_tensor("h_ps", [P, MS, L_DIM], f32))
        g_ps = ctx.enter_context(nc.psum_tensor("g_ps", [P, MS, L_DIM], f32))

        s_v = [ctx.enter_context(nc.semaphore(f"s_v{k}")) for k in range(4)]
        s_xb0 = [ctx.enter_context(nc.semaphore(f"s_xb0k{k}")) for k in range(4)]
        s_u = ctx.enter_context(nc.semaphore("s_u"))
        s_w = ctx.enter_context(nc.semaphore("s_w"))
        s_xc = [ctx.enter_context(nc.semaphore(f"s_xc{i}")) for i in range(NS)]
        s_x8c = [ctx.enter_context(nc.semaphore(f"s_x8c{i}")) for i in range(NS)]
        s_pe = ctx.enter_context(nc.semaphore("s_pe"))
        s_act = ctx.enter_context(nc.semaphore("s_act"))
        s_dve = ctx.enter_context(nc.semaphore("s_dve"))
        s_out = ctx.enter_context(nc.semaphore("s_out"))

        block = ctx.enter_context(nc.Block())

        @block.sync
        def _(sync):
            # Startup stream, in PE consumption order: interleaved
            # (v ko-piece, xb chunk-0 ko-piece) pairs so the first h
            # matmuls start after ~256 KB and never starve.
            sync.dma_start(
                v_sb.ap()[:, 0, :], vt[0]
            ).then_inc(s_v[0], 16)
            sync.dma_start(
                xb_sb.ap()[:, 0, 0, :], xb[0, 0]
            ).then_inc(s_xb0[0], 16)
            sync.dma_start(
                v_sb.ap()[:, 1:3, :], vt[1:3].rearrange("ko p l -> p ko l")
            ).then_inc(s_v[1], 16)
            sync.dma_start(
                xb_sb.ap()[:, 0, 1:3, :],
                xb[0, 1:3].rearrange("ko p m -> p ko m"),
            ).then_inc(s_xb0[1], 16)
            sync.dma_start(
                v_sb.ap()[:, 3:5, :], vt[3:5].rearrange("ko p l -> p ko l")
            ).then_inc(s_v[2], 16)
            sync.dma_start(
                xb_sb.ap()[:, 0, 3:5, :],
                xb[0, 3:5].rearrange("ko p m -> p ko m"),
            ).then_inc(s_xb0[2], 16)
            sync.dma_start(
                v_sb.ap()[:, 5:, :], vt[5:].rearrange("ko p l -> p ko l")
            ).then_inc(s_v[3], 16)
            sync.dma_start(
                xb_sb.ap()[:, 0, 5:, :],
                xb[0, 5:].rearrange("ko p m -> p ko m"),
            ).then_inc(s_xb0[3], 16)
            sync.dma_start(w_sb.ap(), wr[:]).then_inc(s_w, 16)
            sync.dma_start(
                u_sb.ap(), u8.rearrange("ks p ko l -> p ks ko l")
            ).then_inc(s_u, 16)
            sync.dma_start(
                x8_sb.ap()[:, 0],
                x8[0].rearrange("ks p ko m -> p ks ko m"),
            ).then_inc(s_x8c[0], 16)
            sync.dma_start(
                xb_sb.ap()[:, 1], xb[1].rearrange("ko p m -> p ko m")
            ).then_inc(s_xc[1], 16)
            sync.dma_start(
                x8_sb.ap()[:, 1],
                x8[1].rearrange("ks p ko m -> p ks ko m"),
            ).then_inc(s_x8c[1], 16)
            for s in range(2, NS):
                # x slot s%2 free once PE finished chunk s-2
                sync.wait_ge(s_pe, 8 * s - 8)
                sync.dma_start(
                    xb_sb.ap()[:, s % 2],
                    xb[s].rearrange("ko p m -> p ko m"),
                ).then_inc(s_xc[s], 16)
                sync.dma_start(
                    x8_sb.ap()[:, s % 2],
                    x8[s].rearrange("ks p ko m -> p ks ko m"),
                ).then_inc(s_x8c[s], 16)
            sync.wait_ge(s_dve, 3 * MT)
            sync.dma_start(out[:], S_sb.ap()).then_inc(s_out, 16)
            sync.wait_ge(s_out, 16)

        @block.tensor
        def _(tensor):
            # Warm-up: const-broadcast fp32 matmuls (~2 us each cold, no
            # DMA needed) keep the PE busy through the DMA-bound startup
            # so the HAM clock gate reaches 8/8 before the real matmuls.
            c0 = nc.const_aps.aps[(f32, 0.0)]
            c0b = c0.to_broadcast((P, L_DIM))
            for j in range(WARM):
                nc.tensor.matmul(
                    g_ps.ap()[:1, 3, :], c0, c0b, start=True, stop=True
                )
            # ---- chunk 0: ko-outer ----
            xc = xb_sb.ap()[:, 0]
            for ko in range(KO):
                kp = min((ko + 1) // 2, 3)
                tensor.wait_ge(s_v[kp], 16)
                tensor.wait_ge(s_xb0[kp], 16)
                for q in range(MS):
                    mm = nc.tensor.matmul(
                        h_ps.ap()[:, q, :],
                        xc[:, ko, q * P : (q + 1) * P],
                        v_sb.ap()[:, ko, :],
                        start=(ko == 0),
                        stop=(ko == KO - 1),
                    )
                    if ko == KO - 1:
                        mm.then_inc(s_pe, 1)  # ticks 1..4
            tensor.wait_ge(s_u, 16)
            tensor.wait_ge(s_x8c[0], 16)
            x8c = x8_sb.ap()[:, 0]
            for ks in range(KS):
                for q in range(MS):
                    mm = nc.tensor.matmul(
                        g_ps.ap()[:, q, :],
                        x8c[:, ks, :, q * P : (q + 1) * P],
                        u_sb.ap()[:, ks],
                        start=(ks == 0),
                        stop=(ks == KS - 1),
                        perf_mode=DR,
                    )
                    if ks == KS - 1:
                        mm.then_inc(s_pe, 1)  # ticks 5..8
            # ---- steady tiles ----
            for t in range(MS, MT):
                s, q = divmod(t, MS)
                xq = xb_sb.ap()[:, s % 2]
                x8q = x8_sb.ap()[:, s % 2]
                # h bank t%4 free once tanh(t-4) done
                tensor.wait_ge(s_act, 2 * (t - MS) + 1)
                if q == 0:
                    tensor.wait_ge(s_xc[s], 16)
                for ko in range(KO):
                    mm = nc.tensor.matmul(
                        h_ps.ap()[:, q, :],
                        xq[:, ko, q * P : (q + 1) * P],
                        v_sb.ap()[:, ko, :],
                        start=(ko == 0),
                        stop=(ko == KO - 1),
                    )
                mm.then_inc(s_pe, 1)  # tick 2t+1
                # g bank t%4 free once sigmoid(t-4) done
                tensor.wait_ge(s_act, 2 * (t - MS) + 2)
                if q == 0:
                    tensor.wait_ge(s_x8c[s], 16)
                for ks in range(KS):
                    mm = nc.tensor.matmul(
                        g_ps.ap()[:, q, :],
                        x8q[:, ks, :, q * P : (q + 1) * P],
                        u_sb.ap()[:, ks],
                        start=(ks == 0),
                        stop=(ks == KS - 1),
                        perf_mode=DR,
                    )
                mm.then_inc(s_pe, 1)  # tick 2t+2

        @block.scalar
        def _(scalar):
            for t in range(MT):
                scalar.wait_ge(s_pe, pe_h(t))
                if t >= MS:
                    scalar.wait_ge(s_dve, 3 * (t - MS) + 1)  # th slot free
                nc.scalar.activation(
                    th_sb.ap()[:, t % MS, :], h_ps.ap()[:, t % MS, :], AF.Tanh
                ).then_inc(s_act, 1)  # tick 2t+1
                scalar.wait_ge(s_pe, pe_g(t))
                if t >= MS:
                    scalar.wait_ge(s_dve, 3 * (t - MS) + 2)  # sg slot free
                nc.scalar.activation(
                    sg_sb.ap()[:, t % MS, :], g_ps.ap()[:, t % MS, :], AF.Sigmoid
                ).then_inc(s_act, 1)  # tick 2t+2

        @block.vector
        def _(vector):
            vector.wait_ge(s_w, 16)
            for t in range(MT):
                vector.wait_ge(s_act, 2 * t + 1)
                if t:
                    vector.wait_ge(s_dve, 3 * t - 1)  # tw WAR (same engine)
                nc.vector.tensor_tensor(
                    tw_sb.ap(), th_sb.ap()[:, t % MS, :], w_sb.ap(), ALU.mult
                ).then_inc(s_dve, 1)  # tick 3t+1
                vector.wait_ge(s_act, 2 * t + 2)
                vector.wait_ge(s_dve, 3 * t + 1)  # tw RAW (same engine)
                nc.vector.tensor_tensor(
                    z_sb.ap(), tw_sb.ap(), sg_sb.ap()[:, t % MS, :], ALU.mult
                ).then_inc(s_dve, 1)  # tick 3t+2
                vector.wait_ge(s_dve, 3 * t + 2)  # z RAW (same engine)
                nc.vector.tensor_reduce(
                    S_sb.ap()[:, t : t + 1],
                    z_sb.ap(),
                    axis=mybir.AxisListType.X,
                    op=ALU.add,
                ).then_inc(s_dve, 1)  # tick 3t+3

    return nc


def _host_inputs(x, v, u, w):
    """Build the per-core input maps (host-side shard + layout prep)."""
    import ml_dtypes

    bf16 = ml_dtypes.bfloat16
    f8 = ml_dtypes.float8_e4m3fn

    x = np.asarray(x, dtype=np.float32)
    v = np.asarray(v, dtype=np.float32)
    u = np.asarray(u, dtype=np.float32)
    w = np.asarray(w, dtype=np.float32).reshape(L_DIM)

    # vt[ko, p, l] = v[ko*128+p, l]
    vt = np.ascontiguousarray(v.reshape(KO, P, L_DIM).astype(bf16))
    # u8[ks, p, ko, l] = u[ks*256+ko*128+p, l]
    u8 = np.ascontiguousarray(
        u.reshape(KS, 2, P, L_DIM).transpose(0, 2, 1, 3).astype(f8)
    )
    wr = np.ascontiguousarray(np.broadcast_to(w, (P, L_DIM)))

    common = {"vt": vt, "u8": u8, "wr": wr}
    in_maps = []
    for c in range(N_CORES):
        xc = x[:, c * B_LOC : (c + 1) * B_LOC, :].reshape(M, IN_DIM)
        xt = np.ascontiguousarray(xc.T)  # [IN_DIM, M] f32
        # xb[c, ko, p, m'] = xt[ko*128+p, c*512+m']
        xbc = np.ascontiguousarray(
            xt.reshape(KO, P, NS, MSP).transpose(2, 0, 1, 3).astype(bf16)
        )
        # x8[c, ks, p, ko, m'] = xt[ks*256+ko*128+p, c*512+m']
        x8c = np.ascontiguousarray(
            xt.reshape(KS, 2, P, NS, MSP).transpose(3, 0, 2, 1, 4).astype(f8)
        )
        in_maps.append({"xb": xbc, "x8": x8c, **common})
    return in_maps


def kernel(x, v, u, w):
    from concourse.bass_utils import run_bass_kernel_spmd

    if "nc" not in _CACHE:
        _CACHE["nc"] = _build_bass()
    nc = _CACHE["nc"]

    in_maps = _host_inputs(x, v, u, w)
    res = run_bass_kernel_spmd(nc, in_maps, core_ids=list(range(N_CORES)))
    _CACHE["last_result"] = res

    # Gather scores and finish the softmax (over instances) on the host.
    parts = []
    for c in range(N_CORES):
        S = res.results[c]["out"]  # [128, 32], score of row m = t*128 + r
        parts.append(S.T.reshape(M).reshape(N_INST, B_LOC))
    scores = np.concatenate(parts, axis=1).astype(np.float64)  # [256, 128]
    scores -= scores.max(axis=0, keepdims=True)
    e = np.exp(scores)
    alpha = e / e.sum(axis=0, keepdims=True)
    return np.ascontiguousarray(alpha[:, :, None].astype(np.float32))
